# revision 21
# baseline (speedup 1.0000x reference)
"""TRN2 Bass/Tile kernel for the cosine-similarity attention block.

Reference math (fp32, single device):
    K = X @ Wk.T + Wk0 ; Q = X @ Wq.T + Wq0          # [N, E]
    Y = (Q @ K.T) / sqrt(max(|Q_m|^2 * |K_n|^2, eps)) # [N, N] cosine sims
    SM = softmax(Y, axis=0)                           # column softmax
    Z = SM @ X                                        # [N, E]

Distribution (8 cores, row-sharded):
  Each core owns M = N/8 rows of Q / output rows of Z. Locally it computes
  its K/Q row blocks (transposed layout, fp16 matmuls, fp32 PSUM accum),
  normalizes rows (norm reduction via ones-matmul, broadcast via kc=1
  matmul), all-gathers the normalized K^T (fp16, 2 MB/rank), computes its
  [M, N] slice of Y^T (as Yt[n, m], fp16 matmuls), exp on ScalarE with
  accum_out producing per-column partial sums for free, all-reduces the
  column sums (32 KB, split in two halves so the first AR overlaps the
  tail of the Y phase), folds 1/colsum into Et (per-partition DVE scale),
  and computes Zt = X^T-panels @ Et (fp16) -> [E, M] output slice.

All matmul accumulation is fp32 in PSUM. fp16 everywhere is safe because
every intermediate is range-bounded (cosines in [-1, 1], exp in [e^-1, e])
and measured end-to-end error is ~4.5e-4 scale-relative absmax. Softmax
skips max-subtraction for the same reason. Cost-model device time ~530 us
vs a ~500 us PE roofline for this decomposition (95% PE occupancy).
"""

import os
from contextlib import ExitStack

import numpy as np

N, E, C = 8192, 1024, 8

_CACHE = {}


def _build_program(n=N, e=E, c=C, solo=False, repeat=1, upto=4):
    """Emit + compile the SPMD Bass program (one NEFF, all cores)."""
    import concourse.bacc as bacc
    import concourse.mybir as mybir
    import concourse.tile as tile

    F32 = mybir.dt.float32
    F16 = mybir.dt.float16
    F32R = mybir.dt.float32r
    AF = mybir.ActivationFunctionType

    m = n // c          # rows per core
    et = e // 128       # e-tiles
    nt = n // 128       # n-tiles
    jt = m // 128       # n-tiles per core block
    mch = [(i, min(512, m - i)) for i in range(0, m, 512)]  # m chunks (<=512)
    mch16 = [(i, min(1024, m - i)) for i in range(0, m, 1024)]  # fp16 moving max
    rg = [list(range(c))]

    nc = bacc.Bacc("TRN2", target_bir_lowering=False, debug=False, num_devices=c)

    xt = nc.dram_tensor("xt", [e, m], F16, kind="ExternalInput")
    wqt = nc.dram_tensor("wqt", [e, e], F16, kind="ExternalInput")
    wkt = nc.dram_tensor("wkt", [e, e], F16, kind="ExternalInput")
    bq = nc.dram_tensor("bq", [et, 128], F32, kind="ExternalInput")
    bk = nc.dram_tensor("bk", [et, 128], F32, kind="ExternalInput")
    xp = nc.dram_tensor("xp", [et, 128, nt, 128], F16, kind="ExternalInput")
    zt = nc.dram_tensor("zt", [e, m], F32, kind="ExternalOutput")

    with ExitStack() as ctx:
        tc = ctx.enter_context(tile.TileContext(nc))

        dram = ctx.enter_context(tc.tile_pool(name="dram", bufs=1, space="DRAM"))
        ag_in = dram.tile([e, m], F16)
        ag_out = dram.tile([c, e, m], F16, addr_space="Shared")

        consts = ctx.enter_context(tc.tile_pool(name="consts", bufs=1))
        ones_k = consts.tile([128, 1], F16)
        ones_m = consts.tile([1, 128], F16)
        nc.vector.memset(ones_k, 1.0)
        nc.vector.memset(ones_m, 1.0)
        bias_q = consts.tile([128, et], F32)
        bias_k = consts.tile([128, et], F32)
        nc.sync.dma_start(bias_q, bq.ap().rearrange("t p -> p t"))
        nc.sync.dma_start(bias_k, bk.ap().rearrange("t p -> p t"))
        eps1 = consts.tile([1, 1], F32)
        nc.vector.memset(eps1, 1e-6)
        colsum = consts.tile([128, nt], F32)
        cs_full = consts.tile([128, nt], F32)
        inv_cs = consts.tile([128, nt], F32)

        qn_pool = ctx.enter_context(tc.tile_pool(name="qn", bufs=1))
        qnt = [qn_pool.tile([128, m], F16, tag=f"qn{s}", name=f"qn{s}") for s in range(et)]

        # ---------------- Phase 0: projections + row norms ----------------
        with (
            tc.tile_pool(name="p0", bufs=1) as p0,
            tc.tile_pool(name="p0w", bufs=1) as p0w,
            tc.tile_pool(name="p0t", bufs=1) as p0t,
            tc.tile_pool(name="psp", bufs=2, space="PSUM") as psp,
            tc.tile_pool(name="pss", bufs=1, space="PSUM") as pss,
        ):
            xts = []

            def proj(w_handle, bias_sb, out_tiles, dram_out):
                wts = []
                for s in range(et):
                    w_ = p0w.tile([128, e], F16, tag=f"w{s}", name=f"w{s}")
                    nc.sync.dma_start(w_, w_handle.ap()[s * 128:(s + 1) * 128, :])
                    wts.append(w_)
                    if len(xts) <= s:  # first proj: interleave X^T loads
                        x_ = p0.tile([128, m], F16, tag=f"xt{s}", name=f"xts{s}")
                        nc.sync.dma_start(x_, xt.ap()[s * 128:(s + 1) * 128, :])
                        xts.append(x_)
                d_ps = pss.tile([1, m], F32, tag="dps", name="d_ps")
                pf32 = []
                for t in range(et):
                    ps = psp.tile([128, m], F32, tag="pp", name="proj_ps")
                    for s in range(et):
                        lw = wts[s][:, t * 128:(t + 1) * 128]
                        for o, w in mch:
                            nc.tensor.matmul(
                                ps[:, o:o + w],
                                lw,
                                xts[s][:, o:o + w],
                                start=(s == 0),
                                stop=(s == et - 1),
                            )
                    pt = p0.tile([128, m], F32, tag=f"pf{t}", bufs=2, name=f"pf{t}")
                    nc.scalar.activation(pt, ps, AF.Identity, bias=bias_sb[:, t:t + 1])
                    sq = p0t.tile([128, m], F16, tag="sq", bufs=2, name="sq")
                    nc.vector.tensor_mul(sq, pt, pt)
                    for o, w in mch:
                        nc.tensor.matmul(
                            d_ps[0:1, o:o + w],
                            ones_k,
                            sq[:, o:o + w],
                            start=(t == 0),
                            stop=(t == et - 1),
                        )
                    pf32.append(pt)
                dsq = p0t.tile([1, m], F32, tag="dsq", name="dsq")
                nc.scalar.activation(dsq, d_ps, AF.Sqrt, bias=eps1[0:1, 0:1])
                rnorm = p0t.tile([1, m], F32, tag="rn", name="rnorm")
                nc.vector.reciprocal(rnorm, dsq)
                rn16 = p0t.tile([1, m], F16, tag="rn16", name="rn16")
                nc.vector.tensor_copy(rn16, rnorm)
                bc_ps = pss.tile([128, m], F32, tag="bc", name="bc_ps")
                for o, w in mch:
                    nc.tensor.matmul(
                        bc_ps[:, o:o + w],
                        ones_m,
                        rn16[0:1, o:o + w],
                    )
                for t in range(et):
                    nc.vector.tensor_mul(out_tiles[t], pf32[t], bc_ps)
                    if dram_out is not None:
                        nc.sync.dma_start(
                            dram_out[t * 128:(t + 1) * 128, :], out_tiles[t]
                        )

            kn_tiles = [p0.tile([128, m], F16, tag=f"kn{s}", name=f"kn{s}") for s in range(et)]
            proj(wkt, bias_k, kn_tiles, ag_in)
            proj(wqt, bias_q, qnt, None)

        # ---------------- Phase 1: AllGather normalized K^T ----------------
        if not solo:
            nc.gpsimd.collective_compute(
                "AllGather",
                mybir.AluOpType.bypass,
                replica_groups=rg,
                ins=[ag_in.opt()],
                outs=[ag_out.opt()],
            )

        # Et pool opens after phase-0 pools close: its 128 KB/partition and
        # phase 0's transient tiles don't fit SBUF together.
        et_pool = ctx.enter_context(tc.tile_pool(name="etp", bufs=1))
        ets = [et_pool.tile([128, m], F16, tag=f"et{i}", name=f"et{i}") for i in range(nt)]

        # ---------------- Phase 2: Yt = Kn @ Qn^T, exp, column sums --------
        reps = range(repeat if upto >= 2 else 0)
        with (
            tc.tile_pool(name="kp", bufs=2) as kp_pool,
            tc.tile_pool(name="psy", bufs=2, space="PSUM") as psy_pool,
            tc.tile_pool(name="xpp", bufs=2) as xp_pool,
            tc.tile_pool(name="zsb", bufs=2) as z_pool,
            tc.tile_pool(name="psz", bufs=2, space="PSUM") as psz_pool,
          ):
          for rep in reps:
            for cc in range(c):
                kps = []
                for s in range(et):
                    k_ = kp_pool.tile([128, m], F16, tag=f"kp{s}", name=f"kp{s}")
                    src_ap = (ag_in[s * 128:(s + 1) * 128, :] if solo
                              else ag_out[cc, s * 128:(s + 1) * 128, :])
                    nc.sync.dma_start(k_, src_ap)
                    kps.append(k_)
                for j in range(jt):
                    n_t = cc * jt + j
                    psy = psy_pool.tile([128, m], F32, tag="py", name="psy")
                    for s in range(et):
                        lw = kps[s][:, j * 128:(j + 1) * 128]
                        for o, w in mch:
                            nc.tensor.matmul(
                                psy[:, o:o + w],
                                lw,
                                qnt[s][:, o:o + w],
                                start=(s == 0),
                                stop=(s == et - 1),
                            )
                    nc.scalar.activation(
                        ets[n_t], psy, AF.Exp,
                        accum_out=colsum[:, n_t:n_t + 1],
                    )

            # ------------ Phase 3: AllReduce column sums, fold into Et -----
            # Split in two halves: the first AR (columns of the first c/2
            # core blocks) overlaps the tail of phase 2.
            if upto < 3:
                continue
            hw_ = nt // 2
            for half in range(2):
                cs_sl = slice(half * hw_, (half + 1) * hw_)
                ar_in = dram.tile([128, hw_], F32, tag=f"ari{rep}{half}",
                                  name=f"ar_in{rep}{half}")
                ar_out = dram.tile([128, hw_], F32, addr_space="Shared",
                                   tag=f"aro{rep}{half}", name=f"ar_out{rep}{half}")
                nc.sync.dma_start(ar_in, colsum[:, cs_sl])
                if not solo:
                    nc.gpsimd.collective_compute(
                        "AllReduce",
                        mybir.AluOpType.add,
                        replica_groups=rg,
                        ins=[ar_in.opt()],
                        outs=[ar_out.opt()],
                    )
                nc.sync.dma_start(cs_full[:, cs_sl], ar_in if solo else ar_out)
                nc.vector.reciprocal(inv_cs[:, cs_sl], cs_full[:, cs_sl])
                for i in range(half * hw_, (half + 1) * hw_):
                    nc.vector.tensor_scalar_mul(ets[i], ets[i], inv_cs[:, i:i + 1])

            # ------------ Phase 4: Zt = sum_n X^T[:, n] @ Et[n] ------------
            if upto < 4:
                continue
            nh = nt // 2
            for t in range(et):
                psz = psz_pool.tile([128, m], F32, tag="pz", name="psz")
                for half in range(2):
                    xpt = xp_pool.tile([128, nh * 128], F16, tag="xp", name="xpt")
                    nc.sync.dma_start(
                        xpt,
                        xp.ap()[t, :, half * nh:(half + 1) * nh, :]
                        .rearrange("p a b -> p (a b)"),
                    )
                    for ii in range(nh):
                        i = half * nh + ii
                        lw = xpt[:, ii * 128:(ii + 1) * 128]
                        for o, w in mch:
                            nc.tensor.matmul(
                                psz[:, o:o + w],
                                lw,
                                ets[i][:, o:o + w],
                                start=(i == 0),
                                stop=(i == nt - 1),
                            )
                zsb = z_pool.tile([128, m], F32, tag="zt", name="zsb")
                nc.scalar.copy(zsb, psz)
                nc.sync.dma_start(zt.ap()[t * 128:(t + 1) * 128, :], zsb)

    nc.compile()
    return nc


def _prep_inputs(X, Wk, Wq, Wk0, Wq0, n=N, e=E, c=C):
    """Host-side sharding/layout prep. Returns per-core input maps."""
    m = n // c
    et = e // 128
    nt = n // 128
    X = np.ascontiguousarray(X, dtype=np.float32)
    wqt = np.ascontiguousarray(Wq.T, dtype=np.float16)
    wkt = np.ascontiguousarray(Wk.T, dtype=np.float16)
    bq = np.ascontiguousarray(Wq0, dtype=np.float32).reshape(et, 128)
    bk = np.ascontiguousarray(Wk0, dtype=np.float32).reshape(et, 128)
    # xp[e_t, p, n_t, cc] = X[n_t*128 + p, e_t*128 + cc], fp16
    xp = np.ascontiguousarray(
        X.astype(np.float16).reshape(nt, 128, et, 128).transpose(2, 1, 0, 3)
    )
    in_maps = []
    for cc in range(c):
        xt_c = np.ascontiguousarray(X[cc * m:(cc + 1) * m].T.astype(np.float16))
        in_maps.append(
            {"xt": xt_c, "wqt": wqt, "wkt": wkt, "bq": bq, "bk": bk, "xp": xp}
        )
    return in_maps


def _run(X, Wk, Wq, Wk0, Wq0, trace=False, n=N, e=E, c=C):
    from concourse import bass_utils

    key = (n, e, c)
    if key not in _CACHE:
        _CACHE[key] = _build_program(n, e, c)
    nc = _CACHE[key]
    in_maps = _prep_inputs(X, Wk, Wq, Wk0, Wq0, n, e, c)
    res = bass_utils.run_bass_kernel_spmd(
        nc, in_maps, core_ids=list(range(c)), trace=trace
    )
    m = n // c
    Z = np.empty((n, e), dtype=np.float32)
    for cc in range(c):
        Z[cc * m:(cc + 1) * m, :] = res.results[cc]["zt"].T
    return Z, res


def kernel(X, Wk, Wq, Wk0, Wq0):
    Z, _ = _run(X, Wk, Wq, Wk0, Wq0)
    return Z
